# revision 2
# baseline (speedup 1.0000x reference)
"""DeepSeek-MoE layer on 8 TRN2 NeuronCores.

Strategy (intermediate-dim parallel, host-side dispatch):
  - Router (x @ gate_w.T, softmax, top-2) computed on host — it *is* the
    sharding decision (~0.02% of total FLOPs).
  - Every core computes a 384-wide I-slice (1/8 of I=3072) of ALL 8
    routed experts over their routed tokens plus BOTH shared experts
    over all 2048 tokens.  Work per core is exactly 1/8 of the total
    MACs regardless of routing skew — no max-expert-load capacity
    penalty (expert-parallel pays max_e n_e on every core).
  - Cores emit partial down-projections (their I-slice's contribution,
    bf16); the host sums the 8 partials in fp32 and applies the combine
    weights / scatter-add.
  - All matmuls bf16 (fp32 PSUM accumulation).

Device kernel layout:
  - Tokens live on the matmul free axis.  x image [6, 128, 6144]:
    routed-sorted columns (token list concat by expert, 4096) followed
    by natural token order for the shared pass (2048).
  - Work is a list of segments: 8 routed experts (col tiles <= 512)
    then 4 shared col tiles of 512 where both shared experts' down
    projections accumulate into the same PSUM bank (6-step accumulate)
    so the S0+S1 sum leaves the chip as one tensor.
  - Weights stream as uniform [128, 2304] bf16 chunks (0.59 MB), 3 per
    routed expert (gate/up/down) + 6 for the shared pair, packed on
    host into their exact SBUF image in consumption order.
  - Per col-tile inner loop: 3 mi-slices of (6 gate MMs, 6 up MMs,
    silu (ACT), mul->bf16 (DVE)), then down proj in two passes of 3
    PSUM banks (4 gate/up + 3 down = 7 <= 8 banks).
  - PSUM drains alternate ACT/DVE; stores alternate the scalar/sync
    HWDGE rings so nothing serializes on one engine.
"""
import os
import sys
import types

import numpy as np
import ml_dtypes

import concourse.bass as bass
import concourse.tile as tile
import concourse.mybir as mybir
from concourse import bacc
from concourse.bass_utils import run_bass_kernel_spmd

# ---- problem constants (DeepSeekMoE: B=2,S=1024,H=768,I=3072,E=8,NS=2,k=2) --
H = 768          # hidden
I = 3072         # intermediate
E = 8            # routed experts
NS = 2           # shared experts
TOP_K = 2
N_CORES = 8
KH = H // 128        # 6 k-tiles over H
ISL = I // N_CORES   # 384: per-core I-slice
MI = ISL // 128      # 3 mi-tiles per core per expert
T_TOT = 2048
RCOLS = T_TOT * TOP_K      # 4096 routed columns
SCOLS = T_TOT              # 2048 shared columns
NCOL = RCOLS + SCOLS       # 6144
CHUNK = KH * ISL           # 2304 elements per partition per weight chunk
N_CHUNKS = 3 * E + 3 * NS  # 30

BF16 = mybir.dt.bfloat16
F32 = mybir.dt.float32
_bf = ml_dtypes.bfloat16


def _install_ntff_hook():
    """Provide antenv.axon_hooks (missing on this image) so trace=True works."""
    if "antenv.axon_hooks" in sys.modules:
        return
    try:
        from trn_agent_boot.trn_boot import _ntff_profile_via_ctypes
        hook = _ntff_profile_via_ctypes("/opt/axon/libaxon_pjrt.so")
    except Exception:
        hook = None
    mod = types.ModuleType("antenv.axon_hooks")
    mod.get_axon_ntff_profile_hook = lambda: hook
    sys.modules["antenv.axon_hooks"] = mod


def _col_tiles(n):
    """Split n columns into tiles <= 512."""
    if n == 0:
        return []
    nt = -(-n // 512)
    base = n // nt
    sizes = [base + (1 if i < n % nt else 0) for i in range(nt)]
    return sizes


def build_nc(plan):
    """Build the SPMD program.

    plan: tuple of per-routed-expert token counts (n_0..n_7).
    """
    nc = bacc.Bacc(None, target_bir_lowering=False)
    X = nc.dram_tensor("x", [KH, 128, NCOL], BF16, kind="ExternalInput")
    WCH = nc.dram_tensor("wch", [N_CHUNKS, 128, CHUNK], BF16,
                         kind="ExternalInput")
    Y = nc.dram_tensor("y", [H, NCOL], BF16, kind="ExternalOutput")

    # segments: (x_col_offset, n_cols, [(chunk_base, n_experts_in_tile)])
    # routed expert e -> chunks 3e..3e+2; shared pair -> chunks 24..29.
    segs = []
    off = 0
    for e in range(E):
        for n in _col_tiles(plan[e]):
            segs.append(("r", e, off, n))
            off += n
    assert off == RCOLS
    for t in range(4):
        segs.append(("s", t, RCOLS + t * 512, 512))

    with tile.TileContext(nc) as tc:
        with tc.tile_pool(name="wpool", bufs=13) as wpool, \
             tc.tile_pool(name="xpool", bufs=6) as xpool, \
             tc.tile_pool(name="hpool", bufs=14) as hpool, \
             tc.tile_pool(name="sgpool", bufs=4) as sgpool, \
             tc.tile_pool(name="ystage", bufs=8) as ystage, \
             tc.tile_pool(name="gupool", bufs=4, space="PSUM") as gupool, \
             tc.tile_pool(name="ypool", bufs=3, space="PSUM") as ypool:

            w_tiles = {}     # expert id (0..7 routed, 8/9 shared) -> (g,u,d)
            x_tiles = {}     # segment index -> x tile

            def load_w(eid, split_first=False):
                if eid in w_tiles:
                    return
                base = 3 * eid
                tg = wpool.tile([128, KH, ISL], BF16, tag="w")
                src = WCH[base + 0, :, :].rearrange("p (k m) -> p k m", k=KH)
                if split_first:
                    # two halves so the very first matmul waits on 0.3MB
                    nc.sync.dma_start(out=tg[:, :KH // 2, :],
                                      in_=src[:, :KH // 2, :])
                    nc.sync.dma_start(out=tg[:, KH // 2:, :],
                                      in_=src[:, KH // 2:, :])
                else:
                    nc.sync.dma_start(out=tg, in_=src)
                tu = wpool.tile([128, KH, ISL], BF16, tag="w")
                nc.sync.dma_start(out=tu, in_=WCH[base + 1, :, :]
                                  .rearrange("p (k m) -> p k m", k=KH))
                td = wpool.tile([128, MI, H], BF16, tag="w")
                nc.sync.dma_start(out=td, in_=WCH[base + 2, :, :]
                                  .rearrange("p (j i) -> p j i", j=MI))
                w_tiles[eid] = (tg, tu, td)

            def issue_seg(si):
                kind, e, oc, n = segs[si]
                xt = xpool.tile([128, KH, 512], BF16, tag="x")
                for k in range(KH):
                    nc.scalar.dma_start(out=xt[:, k, :n],
                                        in_=X[k, :, oc:oc + n])
                x_tiles[si] = xt
                if kind == "r":
                    load_w(e, split_first=(si == 0))
                else:
                    load_w(E)      # shared expert 0 slice
                    load_w(E + 1)  # shared expert 1 slice

            drain_ctr = [0]

            def compute_seg(si):
                kind, e, oc, n = segs[si]
                xt = x_tiles.pop(si)
                eids = [e] if kind == "r" else [E, E + 1]
                hs = {}
                for eid in eids:
                    tg, tu, td = w_tiles[eid]
                    for mi in range(MI):
                        g = gupool.tile([128, 512], F32, tag="gu")
                        for k in range(KH):
                            nc.tensor.matmul(
                                g[:, :n], tg[:, k, mi * 128:(mi + 1) * 128],
                                xt[:, k, :n],
                                start=(k == 0), stop=(k == KH - 1))
                        u = gupool.tile([128, 512], F32, tag="gu")
                        for k in range(KH):
                            nc.tensor.matmul(
                                u[:, :n], tu[:, k, mi * 128:(mi + 1) * 128],
                                xt[:, k, :n],
                                start=(k == 0), stop=(k == KH - 1))
                        sg = sgpool.tile([128, 512], F32, tag="sg")
                        nc.scalar.activation(sg[:, :n], g[:, :n],
                                             mybir.ActivationFunctionType.Silu)
                        h = hpool.tile([128, 512], BF16, tag="h")
                        nc.vector.tensor_mul(h[:, :n], sg[:, :n], u[:, :n])
                        hs[(eid, mi)] = h
                nacc = len(eids) * MI
                for half in range(2):
                    ys = [ypool.tile([128, 512], F32, tag="y", name=f"y{t}")
                          for t in range(3)]
                    step = 0
                    for eid in eids:
                        td = w_tiles[eid][2]
                        for k in range(MI):
                            for t in range(3):
                                hj = 3 * half + t
                                nc.tensor.matmul(
                                    ys[t][:, :n],
                                    td[:, k, hj * 128:(hj + 1) * 128],
                                    hs[(eid, k)][:, :n],
                                    start=(step == 0), stop=(step == nacc - 1))
                            step += 1
                    for t in range(3):
                        hj = 3 * half + t
                        st = ystage.tile([128, 512], BF16, tag="yst")
                        if drain_ctr[0] % 2 == 0:
                            nc.scalar.copy(st[:, :n], ys[t][:, :n])
                        else:
                            nc.vector.tensor_copy(st[:, :n], ys[t][:, :n])
                        eng = nc.scalar if drain_ctr[0] % 2 == 0 else nc.sync
                        eng.dma_start(
                            out=Y[hj * 128:(hj + 1) * 128, oc:oc + n],
                            in_=st[:, :n])
                        drain_ctr[0] += 1

            PF = 2  # segment prefetch depth
            for si in range(min(PF, len(segs))):
                issue_seg(si)
            for si in range(len(segs)):
                if si + PF < len(segs):
                    issue_seg(si + PF)
                compute_seg(si)
    nc.finalize()
    return nc


def _chunk_gu(wT, c):
    """[H, I] lhsT-layout weight -> this core's [128, 2304] gate/up chunk.
    chunk[p, k*384 + m] = wT[k*128 + p, c*384 + m]"""
    a = wT[:, c * ISL:(c + 1) * ISL].reshape(KH, 128, ISL)
    return np.ascontiguousarray(a.transpose(1, 0, 2)).reshape(128, CHUNK)


def _chunk_d(dT, c):
    """[I, H] lhsT-layout down weight -> this core's [128, 2304] chunk.
    chunk[p, j*768 + i] = dT[c*384 + j*128 + p, i]"""
    a = dT[c * ISL:(c + 1) * ISL, :].reshape(MI, 128, H)
    return np.ascontiguousarray(a.transpose(1, 0, 2)).reshape(128, CHUNK)


_NC_CACHE = {}


def kernel(hidden_states, gate_w, shared_gate, shared_up, shared_down,
           routed_gate, routed_up, routed_down):
    B, S, _ = hidden_states.shape
    T = B * S
    x = np.asarray(hidden_states, np.float32).reshape(T, H)

    # ---- host router (mirrors reference math; fp64 softmax for stability) --
    logits = x @ np.asarray(gate_w, np.float32).T                    # [T, E]
    lg = logits.astype(np.float64)
    sc = np.exp(lg - lg.max(1, keepdims=True))
    sc /= sc.sum(1, keepdims=True)
    topk_idx = np.argsort(-sc, axis=1, kind="stable")[:, :TOP_K]     # [T, k]
    topk_w = np.take_along_axis(sc, topk_idx, axis=1)
    topk_w = topk_w / (topk_w.sum(1, keepdims=True) + 1e-8)          # [T, k]

    tok_lists = []
    tok_weights = []
    for e in range(E):
        sel = (topk_idx == e)
        toks = np.where(sel.any(1))[0]
        w = (topk_w * sel)[toks].sum(1).astype(np.float32)
        tok_lists.append(toks)
        tok_weights.append(w)
    plan = tuple(len(t) for t in tok_lists)
    assert sum(plan) == RCOLS
    tok_concat = np.concatenate(tok_lists)                 # [4096]
    w_concat = np.concatenate(tok_weights)                 # [4096]

    # ---- x image: routed-sorted cols ++ natural order, [6, 128, 6144] ----
    x_bf = x.astype(_bf)
    ximg = np.empty((KH, 128, NCOL), _bf)
    xrT = x_bf[tok_concat].T                               # [768, 4096]
    ximg[:, :, :RCOLS] = xrT.reshape(KH, 128, RCOLS)
    ximg[:, :, RCOLS:] = x_bf.T.reshape(KH, 128, SCOLS)

    # ---- per-core weight chunk images ------------------------------------
    gT = [np.ascontiguousarray(np.asarray(routed_gate[e], np.float32).T)
          .astype(_bf) for e in range(E)]
    uT = [np.ascontiguousarray(np.asarray(routed_up[e], np.float32).T)
          .astype(_bf) for e in range(E)]
    dT = [np.ascontiguousarray(np.asarray(routed_down[e], np.float32).T)
          .astype(_bf) for e in range(E)]
    sgT = [np.ascontiguousarray(np.asarray(shared_gate[s], np.float32).T)
           .astype(_bf) for s in range(NS)]
    suT = [np.ascontiguousarray(np.asarray(shared_up[s], np.float32).T)
           .astype(_bf) for s in range(NS)]
    sdT = [np.ascontiguousarray(np.asarray(shared_down[s], np.float32).T)
           .astype(_bf) for s in range(NS)]

    in_maps = []
    for c in range(N_CORES):
        wch = np.empty((N_CHUNKS, 128, CHUNK), _bf)
        for e in range(E):
            wch[3 * e + 0] = _chunk_gu(gT[e], c)
            wch[3 * e + 1] = _chunk_gu(uT[e], c)
            wch[3 * e + 2] = _chunk_d(dT[e], c)
        for s in range(NS):
            wch[3 * E + 3 * s + 0] = _chunk_gu(sgT[s], c)
            wch[3 * E + 3 * s + 1] = _chunk_gu(suT[s], c)
            wch[3 * E + 3 * s + 2] = _chunk_d(sdT[s], c)
        in_maps.append({"x": ximg, "wch": wch})

    # ---- build + run on 8 cores -----------------------------------------
    if plan not in _NC_CACHE:
        _NC_CACHE.clear()
        _NC_CACHE[plan] = build_nc(plan)
    nc = _NC_CACHE[plan]

    trace = bool(int(os.environ.get("MOE_TRACE", "0")))
    kw = {}
    if trace:
        _install_ntff_hook()
        kw = dict(trace=True, trace_cores=list(range(N_CORES)))
    res = run_bass_kernel_spmd(nc, in_maps, core_ids=list(range(N_CORES)), **kw)
    if trace:
        print(f"HW exec time: {res.exec_time_ns} ns")

    # ---- host combine: sum I-slice partials, weight, scatter -------------
    acc = np.zeros((H, NCOL), np.float32)
    for c in range(N_CORES):
        acc += res.results[c]["y"].astype(np.float32)
    out = np.zeros((T, H), np.float32)
    np.add.at(out, tok_concat, acc[:, :RCOLS].T * w_concat[:, None])
    out += acc[:, RCOLS:].T / NS
    return out.reshape(B, S, H)
